# revision 10
# baseline (speedup 1.0000x reference)
# CrossAttention (B=2, S=2048, D=1024, H=16, dh=64) on 8 trn2 NeuronCores.
#
# Sharding: 32 (batch, head) units, 4 consecutive units per core (cores 0-3
# work on batch 0, cores 4-7 on batch 1). Each core receives its batch's
# hidden states pre-permuted to the on-chip [128, D/128, S] transposed
# layout, per-head slices of Wq/Wk/Wv/Wo (also pre-permuted); it returns a
# partial output y [2048, 1024] (its heads' contribution to the output
# projection). The host sums the four partials per batch and adds bo
# (tensor-parallel unshard of the output projection).
#
# Device algorithm (per core, 4 units = 2 pairs of heads), all matmuls
# fp32r (full PE rate at N>=256, fp32 accumulation). Triangular software
# pipeline over 512-wide s-block "rounds"; for round r:
#   - DMA hidden^T slice; project V (natural [s, feat] layout, stored as V'
#     with fused ones columns), K^T and Q^T (pair-packed [128, s]).
#   - run every attention cell (qb, kc) with max(qb, kc) == r; a cell is:
#     per pair, 4 key chunks of: S^T = K^T-chunk^T Q^T (two heads
#     row-packed, K=64 each), P^T = exp(S^T/8) (mostly on ACT; a tunable
#     fraction of chunks runs on DVE via the Schraudolph exp bit-trick so
#     ACT doesn't pace the PE during cell-dense stretches), O' += V'^T P^T
#     (M=65: psum rows 0-63 unnormalized out^T, row 64 = softmax
#     denominator via the ones column); O' accumulates across rounds in
#     SBUF.
#   - a cell per round is deferred and interleaved with the next round's
#     projections so the ACT engine stays fed.
# Finalize per q-block (interleaved with the last round's cells):
# reciprocal the denominators into a [2, 512] tile (DVE), broadcast both
# units' recips across partitions with ONE K=2 selector matmul (partitions
# 0-63 get unit A's recip, 64-127 unit B's), normalize with two DVE
# multiplies into a unit-stacked [128, 512] O tile, then output projection
# with K=128 (both units contracted per matmul — half the matmuls of the
# per-unit K=64 form) and DMA out.
#
# PSUM plan (8 banks): S^T tiles 2x[128,1024], PV accumulators 2x[128,512],
# projection/broadcast/output transients 2x[128,512] - dedicated pools so
# the streams don't steal each other's slots.
import os
import sys

import numpy as np

try:
    import concourse.bass as bass
except ImportError:  # harness runs from a fresh dir; repo is on the default path
    sys.path.insert(0, "/opt/trn_rl_repo")
    import concourse.bass as bass

import concourse.bacc as bacc
import concourse.mybir as mybir
import concourse.tile as tile
from concourse.bass import ts, ds
from contextlib import ExitStack

B, S, D = 2, 2048, 1024
HEADS, DIM_HEAD = 16, 64
SCALE = DIM_HEAD**-0.5
N_CORES = 8
UNITS = 4  # (b, h) units per core
PAIRS = 2  # head pairs per core
P = 128
SB = S // 512  # 4 s-blocks of 512
QB = S // 512  # 4 q-blocks of 512
DC = D // P  # 8 contraction chunks for projections
KI = S // P  # 16 key chunks of 128
F32 = mybir.dt.float32
F32R = mybir.dt.float32r
I32 = mybir.dt.int32

# Schraudolph exp-as-bf16-bits trick: exp(x*SCALE) ~= bitcast_bf16(
# int16(A*x + B)). A folds the softmax scale and log2(e) into the bf16
# exponent field; B is the bf16 exponent bias minus the minimax shift
# (~0.0579*2^7) that bounds the relative error of the linear-mantissa
# approximation at ~3%, plus 0.5 to center the trunc-to-int. The bf16
# (not f32r) output dtype keeps walrus' "fp32r consumers need fp32r-
# rounded producers" verifier happy; PV matmuls take the bf16 moving
# operand at the same full PE rate. Only a small tunable fraction of exp
# chunks takes this path (error contribution ~f*3%), relieving ACT.
SCH_A = float(2.0**7 * np.log2(np.e) * SCALE)
SCH_B = float(127 * 2**7 - 0.0579 * 2**7 + 0.5)
BF16 = mybir.dt.bfloat16
I16 = mybir.dt.int16


def build_nc():
    nc = bacc.Bacc("TRN2", target_bir_lowering=False, debug=False)

    hiddent = nc.dram_tensor("hiddent", [P, DC, S], F32R, kind="ExternalInput").ap()
    # weights arrive pre-permuted to the on-chip layout (see shard_inputs),
    # declared fp32r end-to-end so the PE takes the full-rate matmul path
    wqt = nc.dram_tensor("wqt", [P, DC, 256], F32R, kind="ExternalInput").ap()
    wkt = nc.dram_tensor("wkt", [P, DC, 256], F32R, kind="ExternalInput").ap()
    wvt = nc.dram_tensor("wvt", [P, DC, 256], F32R, kind="ExternalInput").ap()
    wot = nc.dram_tensor("wot", [P, PAIRS, D], F32R, kind="ExternalInput").ap()
    y = nc.dram_tensor("y", [S, D], F32, kind="ExternalOutput").ap()

    n_xdve = int(os.environ.get("K_XDVE", "7"))  # every-Nth exp chunk -> DVE
    n_xdve_last = int(os.environ.get("K_XDVE_LAST", "4"))

    with tile.TileContext(nc) as tc, ExitStack() as ctx:
        persist = ctx.enter_context(tc.tile_pool(name="persist", bufs=1))
        pt_pool = ctx.enter_context(
            tc.tile_pool(name="pt", bufs=int(os.environ.get("K_PT", "6")))
        )
        otsb_pool = ctx.enter_context(
            tc.tile_pool(name="otsb", bufs=int(os.environ.get("K_OTSB", "4")))
        )
        rc_pool = ctx.enter_context(tc.tile_pool(name="rc", bufs=2))
        y_pool = ctx.enter_context(tc.tile_pool(name="ysb", bufs=2))
        # PSUM: "st" [128,1024] x2 = 4 banks (S^T tiles); "ot" [128,512] x2 +
        # "cell" [128,512] x2 = 4 banks (projection/broadcast/output
        # transients and PV accumulators).
        st_ps = ctx.enter_context(tc.tile_pool(name="stps", bufs=int(os.environ.get("K_ST", "2")), space="PSUM"))
        ot_ps = ctx.enter_context(
            tc.tile_pool(name="otps", bufs=int(os.environ.get("K_OT", "2")), space="PSUM")
        )
        cell_ps = ctx.enter_context(
            tc.tile_pool(name="cellps", bufs=int(os.environ.get("K_CELL", "2")), space="PSUM")
        )

        # ---- persistent SBUF tensors ----
        KT = persist.tile([P, PAIRS, S], F32R)  # K^T pair-packed
        QT = persist.tile([P, PAIRS, S], F32R)  # Q^T pair-packed
        # V' per (k-chunk, pair): [V_unitA(64) | 1 | V_unitB(64) | 1];
        # each unit's PV is M=65 at base 0: out rows 0-63, sum at row 64.
        # bf16 (not fp32r): PV pairs with the bf16 P^T tiles — the PE rejects
        # mixed 32/16-bit matmul inputs — at the same full rate, and bf16's
        # ~0.2% RMS quantization of V is far inside the 2e-2 gate.
        Vp = persist.tile([P, KI, PAIRS, 130], BF16)
        wq_sb = persist.tile([P, DC, 256], F32R)
        wk_sb = persist.tile([P, DC, 256], F32R)
        wv_a = persist.tile([P, 4, 256], F32R)
        wv_b = persist.tile([P, 4, 256], F32R)
        # Wo pair-stacked: partitions 0-63 = unit 2p rows, 64-127 = unit 2p+1
        wo_sb = persist.tile([P, PAIRS, D], F32R)
        # recip-broadcast operands. DVE/PE partition bases must be 32-aligned,
        # so the two per-unit recips live at partitions 0 and 32 of a [33,512]
        # tile (rows 1-31 stay at the memset value and are zeroed out of the
        # contraction by sel2, whose only nonzero rows are 0 and 32):
        # out[m] = rc[0] for m<64 else rc[32].
        sel2 = persist.tile([33, P], F32R)
        rc2 = persist.tile([33, PAIRS, 512], F32R)
        # O' accumulator: rows 0-63 unnormalized out^T, row 64 = denominator
        acc = persist.tile([65, QB, PAIRS, 2, 512], F32)
        # hidden^T lives only within its round
        ht_pool = ctx.enter_context(tc.tile_pool(name="htp", bufs=int(os.environ.get("K_HT", "2"))))

        # memset can't write fp32r; stage ones in f32 and round via copies
        ones_f32 = persist.tile([P, P], F32)
        nc.vector.memset(ones_f32, 1.0)
        sel_f32 = persist.tile([33, P], F32)
        nc.vector.memset(sel_f32, 0.0)
        nc.vector.tensor_copy(sel_f32[0:1, 0:64], ones_f32[0:1, 0:64])
        nc.vector.tensor_copy(sel_f32[32:33, 64:128], ones_f32[32:33, 0:64])
        nc.vector.tensor_copy(sel2, sel_f32)
        # keep rc2's never-written rows finite so sel2's zeros annihilate them
        rc2_f32 = persist.tile([33, PAIRS, 512], F32)
        nc.vector.memset(rc2_f32, 0.0)
        nc.vector.tensor_copy(rc2, rc2_f32)
        # weight DMAs split across the three DGE queues by first use so no
        # single queue serializes the startup: V weights + Wq + Wo on the
        # gpsimd SWDGE queue, Wk on the ACT HWDGE queue (ACT is idle at
        # t=0), hidden tiles on the SP queue.
        nc.gpsimd.dma_start(wv_a, wvt[:, 0:4, :])
        nc.gpsimd.dma_start(wv_b, wvt[:, 4:8, :])
        nc.gpsimd.dma_start(wq_sb, wqt)
        nc.gpsimd.dma_start(wo_sb, wot)
        nc.scalar.dma_start(wk_sb, wkt)
        for col in (64, 129):
            nc.vector.tensor_copy(
                Vp[:, :, :, col : col + 1],
                ones_f32[:, 0:32].rearrange("p (a b c) -> p a b c", a=KI, b=PAIRS),
            )

        exp_counter = [0]

        def attend_cell(qb, kc, pairs=None, xn=None):
            """Attention for q-block qb against key chunks 4*kc..4*kc+3."""
            xn_eff = xn if xn is not None else n_xdve
            for p in pairs if pairs is not None else range(PAIRS):
                otA = cell_ps.tile([P, 512], F32, tag="ot")
                otB = cell_ps.tile([P, 512], F32, tag="ot")
                for k4 in range(4):
                    ki = kc * 4 + k4
                    stt = st_ps.tile([P, 1024], F32, tag="st")
                    nc.tensor.matmul(
                        stt[:, 0:512],
                        KT[0:64, p, ts(ki, 128)],
                        QT[0:64, p, ts(qb, 512)],
                        start=True,
                        stop=True,
                    )
                    nc.tensor.matmul(
                        stt[:, 512:1024],
                        KT[64:128, p, ts(ki, 128)],
                        QT[64:128, p, ts(qb, 512)],
                        start=True,
                        stop=True,
                    )
                    pt = pt_pool.tile([P, 1024], BF16)
                    exp_counter[0] += 1
                    if xn_eff and exp_counter[0] % xn_eff == 0:
                        # Schraudolph exp on DVE (int16 bf16-bit construction)
                        with nc.allow_low_precision(
                            reason="schraudolph exp, ~3% max rel err on a "
                            "small fraction of chunks"
                        ):
                            nc.vector.tensor_scalar(
                                pt.bitcast(I16),
                                stt,
                                SCH_A,
                                SCH_B,
                                mybir.AluOpType.mult,
                                mybir.AluOpType.add,
                            )
                    else:
                        nc.scalar.activation(
                            pt, stt, mybir.ActivationFunctionType.Exp, scale=SCALE
                        )
                    nc.tensor.matmul(
                        otA[0:65, :],
                        Vp[:, ki, p, 0:65],
                        pt[:, 0:512],
                        start=(k4 == 0),
                        stop=(k4 == 3),
                    )
                    nc.tensor.matmul(
                        otB[0:65, :],
                        Vp[:, ki, p, 65:130],
                        pt[:, 512:1024],
                        start=(k4 == 0),
                        stop=(k4 == 3),
                    )
                for u, ot in ((0, otA), (1, otB)):
                    sl = acc[:, qb, p, u, :]
                    if kc == 0:
                        nc.vector.tensor_copy(sl, ot[0:65, :])
                    else:
                        nc.vector.tensor_add(sl, sl, ot[0:65, :])

        def finalize(qb):
            """Normalize q-block qb and run its output projection."""
            ot_pairs = []
            for p in range(PAIRS):
                # per-unit denominators -> recips at partitions 0 / 32
                with nc.allow_low_precision(
                    reason="fp32r rounding of softmax scale is plenty"
                ):
                    nc.vector.reciprocal(rc2[0:1, p, :], acc[64:65, qb, p, 0, :])
                    nc.vector.reciprocal(rc2[32:33, p, :], acc[64:65, qb, p, 1, :])
                # one K=33 selector matmul: partitions 0-63 <- recip_A,
                # 64-127 <- recip_B
                bcp = ot_ps.tile([P, 512], F32, tag="ot")
                nc.tensor.matmul(bcp, sel2, rc2[:, p, :], start=True, stop=True)
                # unit-stacked normalized O tile [128, 512]
                otu = otsb_pool.tile([P, 512], F32R)
                nc.vector.tensor_mul(otu[0:64, :], acc[0:64, qb, p, 0, :], bcp[0:64, :])
                nc.vector.tensor_mul(
                    otu[64:128, :], acc[0:64, qb, p, 1, :], bcp[64:128, :]
                )
                ot_pairs.append(otu)
            for qt_i in range(4):
                for oh in range(2):
                    yps = ot_ps.tile([P, 512], F32, tag="ot")
                    for pi in range(PAIRS):
                        nc.tensor.matmul(
                            yps,
                            ot_pairs[pi][:, ts(qt_i, 128)],
                            wo_sb[:, pi, ds(oh * 512, 512)],
                            start=(pi == 0),
                            stop=(pi == PAIRS - 1),
                        )
                    ysb = y_pool.tile([P, 512], F32)
                    nc.vector.tensor_copy(ysb, yps)
                    nc.sync.dma_start(
                        y[qb * 512 + qt_i * 128 : qb * 512 + (qt_i + 1) * 128,
                          ds(oh * 512, 512)],
                        ysb,
                    )

        # ---- triangular pipeline: per s-block round, transpose + project,
        # then run every attention cell that just became ready ----
        deferred = []
        for sb in range(SB):
            # two separate half-tiles so the first projection matmuls
            # (dc 0-3) can start as soon as the first half lands (tile
            # pool dependencies are tile-granular)
            hTa = ht_pool.tile([P, 4, 512], F32R, tag="hta")
            hTb = ht_pool.tile([P, 4, 512], F32R, tag="htb")
            nc.sync.dma_start(hTa, hiddent[:, 0:4, ts(sb, 512)])
            nc.sync.dma_start(hTb, hiddent[:, 4:8, ts(sb, 512)])

            def hts(dc):
                return (hTa if dc < 4 else hTb)[:, dc % 4, :]
            for st in range(4):
                s0 = sb * 512 + st * 128
                ki_idx = sb * 4 + st
                # V projection for this s-tile (natural layout, all 4 units)
                vps = ot_ps.tile([P, 512], F32, tag="ot")
                for dc in range(DC):
                    nc.tensor.matmul(
                        vps[:, :256],
                        hts(dc)[:, ts(st, 128)],
                        (wv_a if dc < 4 else wv_b)[:, dc % 4, :],
                        start=(dc == 0),
                        stop=(dc == DC - 1),
                    )
                for p in range(PAIRS):
                    nc.vector.tensor_copy(
                        Vp[:, ki_idx, p, 0:64], vps[:, (2 * p) * 64 : (2 * p + 1) * 64]
                    )
                    nc.vector.tensor_copy(
                        Vp[:, ki_idx, p, 65:129],
                        vps[:, (2 * p + 1) * 64 : (2 * p + 2) * 64],
                    )
            # K^T / Q^T projections for this s-block (pair-packed),
            # interleaved with cells deferred from the previous round so the
            # ACT engine stays fed while the PE runs projections
            projs = [(w, o, p) for (w, o) in ((wk_sb, KT), (wq_sb, QT))
                     for p in range(PAIRS)]
            for i, (w_sb, out_t, p) in enumerate(projs):
                kps = ot_ps.tile([P, 512], F32, tag="ot")
                for dc in range(DC):
                    nc.tensor.matmul(
                        kps,
                        w_sb[:, dc, ts(p, 128)],
                        hts(dc),
                        start=(dc == 0),
                        stop=(dc == DC - 1),
                    )
                nc.vector.tensor_copy(out_t[:, p, ts(sb, 512)], kps)
                if i < len(deferred):
                    attend_cell(*deferred[i])
            deferred = []

            # newly-ready cells: earlier q-blocks against this round's keys,
            # plus this q-block against all keys so far
            new_cells = [(qb, sb) for qb in range(sb)]
            new_cells += [(sb, kc) for kc in range(sb + 1)]
            if sb < SB - 1:
                # defer the last N_DEFER cells, split per pair, to interleave
                # with the next round's projections
                n_defer = min(int(os.environ.get("K_DEFER", "1")), len(new_cells))
                if n_defer:
                    for qb, kc in new_cells[-n_defer:]:
                        for p in range(PAIRS):
                            deferred.append((qb, kc, [p]))
                    deferred = deferred[:4]
                    leftover = [
                        (qb, kc, [p])
                        for (qb, kc) in new_cells[-n_defer:]
                        for p in range(PAIRS)
                    ][4:]
                    new_cells = new_cells[:-n_defer]
                else:
                    leftover = []
                for cell in new_cells:
                    attend_cell(cell[0], cell[1])
                for qb, kc, ps in leftover:
                    attend_cell(qb, kc, ps)
            else:
                # last round: run this q-block's own cells first so its
                # finalize unlocks early, then finalize each q-block one
                # cell after its final cell lands, so finalize PE work
                # fills ACT-wait gaps of the in-flight cell
                if os.environ.get("K_LASTFIRST", "1") == "1":
                    new_cells = new_cells[sb:] + new_cells[:sb]
                done = []
                for i, (qb, kc) in enumerate(new_cells):
                    attend_cell(qb, kc, xn=n_xdve_last)
                    if done:
                        finalize(done.pop())
                    if kc == SB - 1:
                        done.append(qb)
                for qb in done:
                    finalize(qb)
    nc.compile()
    return nc


_NC = None


def get_nc():
    global _NC
    if _NC is None:
        _NC = build_nc()
    return _NC


def shard_inputs(hidden_states, Wq, Wk, Wv, Wo):
    """Per-core input maps. Core c: batch c//4, heads 4*(c%4) .. 4*(c%4)+3."""
    hidden_states = np.asarray(hidden_states, np.float32)
    Wq, Wk, Wv, Wo = (np.asarray(w, np.float32) for w in (Wq, Wk, Wv, Wo))
    in_maps = []
    for c in range(N_CORES):
        b = c // 4
        f0 = (c % 4) * 4 * DIM_HEAD  # first feature row/col of this core's heads
        rows = slice(f0, f0 + UNITS * DIM_HEAD)

        def proj_layout(w):
            # W[rows].T is [D, 256]; on-chip layout is [128, DC, 256]
            return np.ascontiguousarray(
                w[rows, :].T.reshape(DC, P, 256).transpose(1, 0, 2)
            )

        # Wo[:, rows].T is [256, D]; on-chip layout [128, PAIRS, D] stacks
        # each pair's two units on the partition axis (K=128 out-proj)
        wot = np.ascontiguousarray(
            Wo[:, rows].T.reshape(PAIRS, P, D).transpose(1, 0, 2)
        )
        in_maps.append(
            {
                "hiddent": np.ascontiguousarray(
                    hidden_states[b].T.reshape(DC, P, S).transpose(1, 0, 2)
                ),
                "wqt": proj_layout(Wq),
                "wkt": proj_layout(Wk),
                "wvt": proj_layout(Wv),
                "wot": wot,
            }
        )
    return in_maps


def unshard_outputs(results, bo):
    out = np.zeros((B, S, D), np.float32)
    for c, res in enumerate(results):
        out[c // 4] += res["y"]
    out += np.asarray(bo, np.float32)[None, None, :]
    return out


def kernel(hidden_states, Wq, Wk, Wv, Wo, bo, _trace=False):
    from concourse.bass_utils import run_bass_kernel_spmd

    nc = get_nc()
    in_maps = shard_inputs(hidden_states, Wq, Wk, Wv, Wo)
    res = run_bass_kernel_spmd(nc, in_maps, list(range(N_CORES)), trace=_trace)
    out = unshard_outputs(res.results, bo)
    if _trace:
        return out, res
    return out


# revision 22
# speedup vs baseline: 1.1100x; 1.1100x over previous
# CrossAttention (B=2, S=2048, D=1024, H=16, dh=64) on 8 trn2 NeuronCores.
#
# Sharding: 32 (batch, head) units, 4 consecutive units per core (cores 0-3
# work on batch 0, cores 4-7 on batch 1). Each core receives its batch's
# hidden states pre-permuted to the on-chip [128, D/128, S] transposed
# layout, per-head slices of Wq/Wk/Wv/Wo (also pre-permuted); it returns a
# partial output y [2048, 1024] (its heads' contribution to the output
# projection). The host sums the four partials per batch and adds bo
# (tensor-parallel unshard of the output projection).
#
# Device algorithm (per core, 4 units = 2 pairs of heads), all matmuls
# fp32r (full PE rate at N>=256, fp32 accumulation). Triangular software
# pipeline over 512-wide s-block "rounds"; for round r:
#   - DMA hidden^T slice; project V (natural [s, feat] layout, stored as V'
#     with fused ones columns), K^T and Q^T (pair-packed [128, s]).
#   - run every attention cell (qb, kc) with max(qb, kc) == r; a cell is:
#     per pair, 4 key chunks of: S^T = K^T-chunk^T Q^T (two heads
#     row-packed, K=64 each), P^T = exp(S^T/8) (mostly on ACT; a tunable
#     fraction of chunks runs on DVE via the Schraudolph exp bit-trick so
#     ACT doesn't pace the PE during cell-dense stretches), O' += V'^T P^T
#     (M=65: psum rows 0-63 unnormalized out^T, row 64 = softmax
#     denominator via the ones column); O' accumulates across rounds in
#     SBUF.
#   - a cell per round is deferred and interleaved with the next round's
#     projections so the ACT engine stays fed.
# Finalize per q-block (interleaved with the last round's cells):
# reciprocal the denominators into a [2, 512] tile (DVE), broadcast both
# units' recips across partitions with ONE K=2 selector matmul (partitions
# 0-63 get unit A's recip, 64-127 unit B's), normalize with two DVE
# multiplies into a unit-stacked [128, 512] O tile, then output projection
# with K=128 (both units contracted per matmul — half the matmuls of the
# per-unit K=64 form) and DMA out.
#
# PSUM plan (8 banks): S^T tiles 2x[128,1024], PV accumulators 2x[128,512],
# projection/broadcast/output transients 2x[128,512] - dedicated pools so
# the streams don't steal each other's slots.
import os
import sys

import numpy as np

try:
    import concourse.bass as bass
except ImportError:  # harness runs from a fresh dir; repo is on the default path
    sys.path.insert(0, "/opt/trn_rl_repo")
    import concourse.bass as bass

import concourse.bacc as bacc
import concourse.mybir as mybir
import concourse.tile as tile
from concourse.bass import ts, ds
from contextlib import ExitStack

B, S, D = 2, 2048, 1024
HEADS, DIM_HEAD = 16, 64
SCALE = DIM_HEAD**-0.5
N_CORES = 8
UNITS = 4  # (b, h) units per core
PAIRS = 2  # head pairs per core
P = 128
SB = S // 512  # 4 s-blocks of 512
QB = S // 512  # 4 q-blocks of 512
DC = D // P  # 8 contraction chunks for projections
KI = S // P  # 16 key chunks of 128
F32 = mybir.dt.float32
F32R = mybir.dt.float32r
I32 = mybir.dt.int32

# Schraudolph exp-as-bf16-bits trick: exp(x*SCALE) ~= bitcast_bf16(
# int16(A*x + B)). A folds the softmax scale and log2(e) into the bf16
# exponent field; B is the bf16 exponent bias minus the minimax shift
# (~0.0579*2^7) that bounds the relative error of the linear-mantissa
# approximation at ~3%, plus 0.5 to center the trunc-to-int. The bf16
# (not f32r) output dtype keeps walrus' "fp32r consumers need fp32r-
# rounded producers" verifier happy; PV matmuls take the bf16 moving
# operand at the same full PE rate. Only a small tunable fraction of exp
# chunks takes this path (error contribution ~f*3%), relieving ACT.
SCH_A = float(2.0**7 * np.log2(np.e) * SCALE)
SCH_B = float(127 * 2**7 - 0.0579 * 2**7 + 0.5)
BF16 = mybir.dt.bfloat16
I16 = mybir.dt.int16


def build_nc():
    nc = bacc.Bacc("TRN2", target_bir_lowering=False, debug=False)

    hiddent = nc.dram_tensor("hiddent", [P, DC, S], F32R, kind="ExternalInput").ap()
    # weights arrive pre-permuted to the on-chip layout (see shard_inputs),
    # declared fp32r end-to-end so the PE takes the full-rate matmul path
    wqt = nc.dram_tensor("wqt", [P, DC, 256], F32R, kind="ExternalInput").ap()
    wkt = nc.dram_tensor("wkt", [P, DC, 256], F32R, kind="ExternalInput").ap()
    wvt = nc.dram_tensor("wvt", [P, DC, 256], F32R, kind="ExternalInput").ap()
    wot = nc.dram_tensor("wot", [P, PAIRS, D], F32R, kind="ExternalInput").ap()
    y = nc.dram_tensor("y", [S, D], F32, kind="ExternalOutput").ap()

    n_xdve = int(os.environ.get("K_XDVE", "0"))  # every-Nth exp chunk -> DVE
    n_xdve_last = int(os.environ.get("K_XDVE_LAST", "0"))

    with tile.TileContext(nc) as tc, ExitStack() as ctx:
        persist = ctx.enter_context(tc.tile_pool(name="persist", bufs=1))
        pt_pool = ctx.enter_context(
            tc.tile_pool(name="pt", bufs=int(os.environ.get("K_PT", "6")))
        )
        otsb_pool = ctx.enter_context(
            tc.tile_pool(name="otsb", bufs=int(os.environ.get("K_OTSB", "4")))
        )
        rc_pool = ctx.enter_context(tc.tile_pool(name="rc", bufs=2))
        y_pool = ctx.enter_context(tc.tile_pool(name="ysb", bufs=2))
        # PSUM: "st" [128,1024] x2 = 4 banks (S^T tiles); "ot" [128,512] x2 +
        # "cell" [128,512] x2 = 4 banks (projection/broadcast/output
        # transients and PV accumulators).
        st_ps = ctx.enter_context(tc.tile_pool(name="stps", bufs=int(os.environ.get("K_ST", "2")), space="PSUM"))
        ot_ps = ctx.enter_context(
            tc.tile_pool(name="otps", bufs=int(os.environ.get("K_OT", "2")), space="PSUM")
        )
        cell_ps = ctx.enter_context(
            tc.tile_pool(name="cellps", bufs=int(os.environ.get("K_CELL", "2")), space="PSUM")
        )

        # ---- persistent SBUF tensors ----
        KT = persist.tile([P, PAIRS, S], F32R)  # K^T pair-packed
        QT = persist.tile([P, PAIRS, S], F32R)  # Q^T pair-packed
        # V' per (k-chunk, pair): [V_unitA(64) | 1 | V_unitB(64) | 1];
        # each unit's PV is M=65 at base 0: out rows 0-63, sum at row 64.
        # bf16 (not fp32r): PV pairs with the bf16 P^T tiles — the PE rejects
        # mixed 32/16-bit matmul inputs — at the same full rate, and bf16's
        # ~0.2% RMS quantization of V is far inside the 2e-2 gate.
        Vp = persist.tile([P, KI, PAIRS, 130], BF16)
        wq_sb = persist.tile([P, DC, 256], F32R)
        wk_sb = persist.tile([P, DC, 256], F32R)
        wv_a = persist.tile([P, 4, 256], F32R)
        wv_b = persist.tile([P, 4, 256], F32R)
        # Wo pair-stacked: partitions 0-63 = unit 2p rows, 64-127 = unit 2p+1
        wo_sb = persist.tile([P, PAIRS, D], F32R)
        # recip-broadcast operands. DVE/PE partition bases must be 32-aligned,
        # so the two per-unit recips live at partitions 0 and 32 of a [33,512]
        # tile (rows 1-31 stay at the memset value and are zeroed out of the
        # contraction by sel2, whose only nonzero rows are 0 and 32):
        # out[m] = rc[0] for m<64 else rc[32].
        sel2 = persist.tile([33, P], F32R)
        rc2 = persist.tile([33, 2, PAIRS, 512], F32R)  # dim1: qb%2 slot
        # O' accumulator: rows 0-63 unnormalized out^T, row 64 = denominator
        acc = persist.tile([65, QB, PAIRS, 2, 512], F32)
        # hidden^T lives only within its round
        ht_pool = ctx.enter_context(tc.tile_pool(name="htp", bufs=int(os.environ.get("K_HT", "2"))))

        # memset can't write fp32r; stage ones in f32 and round via copies
        ones_f32 = persist.tile([P, P], F32)
        nc.vector.memset(ones_f32, 1.0)
        sel_f32 = persist.tile([33, P], F32)
        nc.vector.memset(sel_f32, 0.0)
        nc.vector.tensor_copy(sel_f32[0:1, 0:64], ones_f32[0:1, 0:64])
        nc.vector.tensor_copy(sel_f32[32:33, 64:128], ones_f32[32:33, 0:64])
        nc.vector.tensor_copy(sel2, sel_f32)
        # keep rc2's never-written rows finite so sel2's zeros annihilate them
        rc2_f32 = persist.tile([33, 2, PAIRS, 512], F32)
        nc.vector.memset(rc2_f32, 0.0)
        nc.vector.tensor_copy(rc2, rc2_f32)
        # weight DMAs split across the three DGE queues by first use so no
        # single queue serializes the startup: V weights + Wq + Wo on the
        # gpsimd SWDGE queue, Wk on the ACT HWDGE queue (ACT is idle at
        # t=0), hidden tiles on the SP queue.
        nc.gpsimd.dma_start(wv_a, wvt[:, 0:4, :])
        nc.gpsimd.dma_start(wv_b, wvt[:, 4:8, :])
        nc.gpsimd.dma_start(wq_sb, wqt)
        nc.gpsimd.dma_start(wo_sb, wot)
        nc.scalar.dma_start(wk_sb, wkt)
        for col in (64, 129):
            nc.vector.tensor_copy(
                Vp[:, :, :, col : col + 1],
                ones_f32[:, 0:32].rearrange("p (a b c) -> p a b c", a=KI, b=PAIRS),
            )

        exp_counter = [0]
        chain_tiles = {}  # (qb, p) -> (otA, otB) for multi-cell PSUM chains

        def attend_cell(qb, kc, pairs=None, xn=None, first=True, last=True):
            """Attention for q-block qb against key chunks 4*kc..4*kc+3.

            first/last: diagonal cells of a round share one PSUM accumulation
            group across kc (the PE accumulates for free), so only the
            chain's last cell touches the SBUF accumulator — one copy/add
            per chain instead of one per cell.
            """
            xn_eff = xn if xn is not None else n_xdve
            for p in pairs if pairs is not None else range(PAIRS):
                if first:
                    otA = cell_ps.tile([P, 512], F32, tag="ot")
                    otB = cell_ps.tile([P, 512], F32, tag="ot")
                    chain_tiles[(qb, p)] = (otA, otB)
                else:
                    otA, otB = chain_tiles[(qb, p)]
                for k4 in range(4):
                    ki = kc * 4 + k4
                    stt = st_ps.tile([P, 1024], F32, tag="st")
                    nc.tensor.matmul(
                        stt[:, 0:512],
                        KT[0:64, p, ts(ki, 128)],
                        QT[0:64, p, ts(qb, 512)],
                        start=True,
                        stop=True,
                    )
                    nc.tensor.matmul(
                        stt[:, 512:1024],
                        KT[64:128, p, ts(ki, 128)],
                        QT[64:128, p, ts(qb, 512)],
                        start=True,
                        stop=True,
                    )
                    pt = pt_pool.tile([P, 1024], BF16)
                    exp_counter[0] += 1
                    if xn_eff and exp_counter[0] % xn_eff == 0:
                        # Schraudolph exp on DVE (int16 bf16-bit construction)
                        with nc.allow_low_precision(
                            reason="schraudolph exp, ~3% max rel err on a "
                            "small fraction of chunks"
                        ):
                            nc.vector.tensor_scalar(
                                pt.bitcast(I16),
                                stt,
                                SCH_A,
                                SCH_B,
                                mybir.AluOpType.mult,
                                mybir.AluOpType.add,
                            )
                    else:
                        nc.scalar.activation(
                            pt, stt, mybir.ActivationFunctionType.Exp, scale=SCALE
                        )
                    nc.tensor.matmul(
                        otA[0:65, :],
                        Vp[:, ki, p, 0:65],
                        pt[:, 0:512],
                        start=(k4 == 0 and first),
                        stop=(k4 == 3 and last),
                    )
                    nc.tensor.matmul(
                        otB[0:65, :],
                        Vp[:, ki, p, 65:130],
                        pt[:, 512:1024],
                        start=(k4 == 0 and first),
                        stop=(k4 == 3 and last),
                    )
                if last:
                    for u, ot in ((0, otA), (1, otB)):
                        sl = acc[:, qb, p, u, :]
                        if first and kc == 0:
                            # single-cell chain at kc 0: plain copy
                            nc.vector.tensor_copy(sl, ot[0:65, :])
                        elif not first:
                            # diagonal chain starting at kc 0: copy replaces
                            # the whole add series
                            nc.vector.tensor_copy(sl, ot[0:65, :])
                        else:
                            nc.vector.tensor_add(sl, sl, ot[0:65, :])

        def finalize_recips(qb, p):
            """Stage 1 (DVE only): pair p's per-unit denominator reciprocals."""
            sl = qb % 2
            with nc.allow_low_precision(
                reason="fp32r rounding of softmax scale is plenty"
            ):
                nc.vector.reciprocal(rc2[0:1, sl, p, :], acc[64:65, qb, p, 0, :])
                nc.vector.reciprocal(rc2[32:33, sl, p, :], acc[64:65, qb, p, 1, :])

        def finalize_norm(qb, p):
            """Stage 2: recip broadcast (one K=33 selector matmul: partitions
            0-63 <- recip_A, 64-127 <- recip_B) + DVE normalize into a
            unit-stacked [128, 512] O tile for pair p."""
            sl = qb % 2
            bcp = ot_ps.tile([P, 512], F32, tag="ot")
            nc.tensor.matmul(bcp, sel2, rc2[:, sl, p, :], start=True, stop=True)
            otu = otsb_pool.tile([P, 512], F32R)
            nc.vector.tensor_mul(otu[0:64, :], acc[0:64, qb, p, 0, :], bcp[0:64, :])
            nc.vector.tensor_mul(otu[64:128, :], acc[0:64, qb, p, 1, :], bcp[64:128, :])
            return otu

        def finalize_outproj(qb, ot_pairs, groups, drain=False):
            """Stage 3 (PE heavy): output projection + DMA out for the given
            (qt, oh) groups. In the drain (no cells left) the PSUM->SBUF
            staging copies alternate ACT/DVE and the stores alternate the
            SP/ACT DMA queues so no single engine serializes the tail."""
            for gi, (qt_i, oh) in enumerate(groups):
                yps = ot_ps.tile([P, 512], F32, tag="ot")
                for pi in range(PAIRS):
                    nc.tensor.matmul(
                        yps,
                        ot_pairs[pi][:, ts(qt_i, 128)],
                        wo_sb[:, pi, ds(oh * 512, 512)],
                        start=(pi == 0),
                        stop=(pi == PAIRS - 1),
                    )
                ysb = y_pool.tile([P, 512], F32)
                ydst = y[qb * 512 + qt_i * 128 : qb * 512 + (qt_i + 1) * 128,
                         ds(oh * 512, 512)]
                if drain and gi % 2 == 0:
                    nc.scalar.copy(ysb, yps)
                    nc.scalar.dma_start(ydst, ysb)
                else:
                    nc.vector.tensor_copy(ysb, yps)
                    nc.sync.dma_start(ydst, ysb)

        GROUPS_A = [(0, 0), (0, 1), (1, 0), (1, 1)]
        GROUPS_B = [(2, 0), (2, 1), (3, 0), (3, 1)]

        def finalize(qb):
            for p in range(PAIRS):
                finalize_recips(qb, p)
            ot_pairs = [finalize_norm(qb, p) for p in range(PAIRS)]
            finalize_outproj(qb, ot_pairs, GROUPS_A + GROUPS_B)

        # ---- triangular pipeline: per s-block round, transpose + project,
        # then run every attention cell that just became ready ----
        deferred = []
        for sb in range(SB):
            # two separate half-tiles so the first projection matmuls
            # (dc 0-3) can start as soon as the first half lands (tile
            # pool dependencies are tile-granular)
            hTa = ht_pool.tile([P, 4, 512], F32R, tag="hta")
            hTb = ht_pool.tile([P, 4, 512], F32R, tag="htb")
            nc.sync.dma_start(hTa, hiddent[:, 0:4, ts(sb, 512)])
            nc.sync.dma_start(hTb, hiddent[:, 4:8, ts(sb, 512)])

            def hts(dc):
                return (hTa if dc < 4 else hTb)[:, dc % 4, :]
            for st in range(4):
                s0 = sb * 512 + st * 128
                ki_idx = sb * 4 + st
                # V projection for this s-tile (natural layout, all 4 units)
                vps = ot_ps.tile([P, 512], F32, tag="ot")
                for dc in range(DC):
                    nc.tensor.matmul(
                        vps[:, :256],
                        hts(dc)[:, ts(st, 128)],
                        (wv_a if dc < 4 else wv_b)[:, dc % 4, :],
                        start=(dc == 0),
                        stop=(dc == DC - 1),
                    )
                for p in range(PAIRS):
                    nc.vector.tensor_copy(
                        Vp[:, ki_idx, p, 0:64], vps[:, (2 * p) * 64 : (2 * p + 1) * 64]
                    )
                    nc.vector.tensor_copy(
                        Vp[:, ki_idx, p, 65:129],
                        vps[:, (2 * p + 1) * 64 : (2 * p + 2) * 64],
                    )
            # K^T / Q^T projections for this s-block (pair-packed),
            # interleaved with cells deferred from the previous round so the
            # ACT engine stays fed while the PE runs projections
            projs = [(w, o, p) for (w, o) in ((wk_sb, KT), (wq_sb, QT))
                     for p in range(PAIRS)]
            for i, (w_sb, out_t, p) in enumerate(projs):
                kps = ot_ps.tile([P, 512], F32, tag="ot")
                for dc in range(DC):
                    nc.tensor.matmul(
                        kps,
                        w_sb[:, dc, ts(p, 128)],
                        hts(dc),
                        start=(dc == 0),
                        stop=(dc == DC - 1),
                    )
                nc.vector.tensor_copy(out_t[:, p, ts(sb, 512)], kps)
                if i < len(deferred):
                    dq, dk, dp, dfirst, dlast = deferred[i]
                    attend_cell(dq, dk, dp, first=dfirst, last=dlast)
            deferred = []

            # newly-ready work as per-pair "pieces": off-diagonal cells
            # (qb, sb) as single-cell groups, then this round's diagonal
            # (sb, 0..sb) as a pair-major PSUM chain (one accumulation group
            # per pair spanning all its kc — no per-cell SBUF adds).
            pieces = [(qb, sb, [p], True, True)
                      for qb in range(sb) for p in range(PAIRS)]
            for p in range(PAIRS):
                for kc in range(sb + 1):
                    pieces.append((sb, kc, [p], kc == 0, kc == sb))
            if sb < SB - 1:
                # defer the last N_DEFER pieces to interleave with the next
                # round's projections
                n_defer = min(int(os.environ.get("K_DEFER", "2")), len(pieces), 4)
                deferred = pieces[len(pieces) - n_defer:] if n_defer else []
                for piece in pieces[: len(pieces) - n_defer]:
                    attend_cell(piece[0], piece[1], piece[2],
                                first=piece[3], last=piece[4])
            else:
                # last round: staged finalize via a thunk queue. Recips (DVE
                # only, no PE impact) fire the moment a (qb, pair) completes;
                # PE-bearing stages (norm's bcp matmul, outproj 4-group
                # halves) are queued and popped between pieces so the PE
                # never sits in-order behind an unfinished DVE leg.
                from collections import deque

                # per-(qb, pair) outstanding piece counts
                left = {}
                for piece in pieces:
                    for p in piece[2]:
                        left[(piece[0], p)] = left.get((piece[0], p), 0) + 1
                thunks = deque()
                fin_state = {}  # qb -> {p: otu}

                def push_finalize(qb, p):
                    finalize_recips(qb, p)  # immediate: DVE-only

                    def norm_thunk(drain=False, qb=qb, p=p):
                        fin_state.setdefault(qb, {})[p] = finalize_norm(qb, p)

                    thunks.append(norm_thunk)
                    if p == PAIRS - 1:
                        def oa(drain=False, qb=qb):
                            ots = [fin_state[qb][0], fin_state[qb][1]]
                            finalize_outproj(qb, ots, GROUPS_A, drain=drain)

                        def ob(drain=False, qb=qb):
                            ots = [fin_state[qb][0], fin_state[qb][1]]
                            finalize_outproj(qb, ots, GROUPS_B, drain=drain)

                        thunks.append(oa)
                        thunks.append(ob)

                for piece in pieces:
                    attend_cell(piece[0], piece[1], piece[2], xn=n_xdve_last,
                                first=piece[3], last=piece[4])
                    if thunks:
                        thunks.popleft()()
                    for p in piece[2]:
                        left[(piece[0], p)] -= 1
                        if left[(piece[0], p)] == 0:
                            push_finalize(piece[0], p)
                while thunks:
                    thunks.popleft()(True)
    nc.compile()
    return nc


_NC = None


def get_nc():
    global _NC
    if _NC is None:
        _NC = build_nc()
    return _NC


def shard_inputs(hidden_states, Wq, Wk, Wv, Wo):
    """Per-core input maps. Core c: batch c//4, heads 4*(c%4) .. 4*(c%4)+3."""
    hidden_states = np.asarray(hidden_states, np.float32)
    Wq, Wk, Wv, Wo = (np.asarray(w, np.float32) for w in (Wq, Wk, Wv, Wo))
    in_maps = []
    for c in range(N_CORES):
        b = c // 4
        f0 = (c % 4) * 4 * DIM_HEAD  # first feature row/col of this core's heads
        rows = slice(f0, f0 + UNITS * DIM_HEAD)

        def proj_layout(w):
            # W[rows].T is [D, 256]; on-chip layout is [128, DC, 256]
            return np.ascontiguousarray(
                w[rows, :].T.reshape(DC, P, 256).transpose(1, 0, 2)
            )

        # Wo[:, rows].T is [256, D]; on-chip layout [128, PAIRS, D] stacks
        # each pair's two units on the partition axis (K=128 out-proj)
        wot = np.ascontiguousarray(
            Wo[:, rows].T.reshape(PAIRS, P, D).transpose(1, 0, 2)
        )
        in_maps.append(
            {
                "hiddent": np.ascontiguousarray(
                    hidden_states[b].T.reshape(DC, P, S).transpose(1, 0, 2)
                ),
                "wqt": proj_layout(Wq),
                "wkt": proj_layout(Wk),
                "wvt": proj_layout(Wv),
                "wot": wot,
            }
        )
    return in_maps


def unshard_outputs(results, bo):
    out = np.zeros((B, S, D), np.float32)
    for c, res in enumerate(results):
        out[c // 4] += res["y"]
    out += np.asarray(bo, np.float32)[None, None, :]
    return out


def kernel(hidden_states, Wq, Wk, Wv, Wo, bo, _trace=False):
    from concourse.bass_utils import run_bass_kernel_spmd

    nc = get_nc()
    in_maps = shard_inputs(hidden_states, Wq, Wk, Wv, Wo)
    res = run_bass_kernel_spmd(nc, in_maps, list(range(N_CORES)), trace=_trace)
    out = unshard_outputs(res.results, bo)
    if _trace:
        return out, res
    return out


# revision 30
# speedup vs baseline: 1.1596x; 1.0447x over previous
# CrossAttention (B=2, S=2048, D=1024, H=16, dh=64) on 8 trn2 NeuronCores.
#
# Sharding: 32 (batch, head) units, 4 consecutive units per core (cores 0-3
# work on batch 0, cores 4-7 on batch 1). Each core receives its batch's
# hidden states pre-permuted to the on-chip [128, D/128, S] transposed
# layout, per-head slices of Wq/Wk/Wv/Wo (also pre-permuted); it returns a
# partial output y [2048, 1024] (its heads' contribution to the output
# projection). The host sums the four partials per batch and adds bo
# (tensor-parallel unshard of the output projection).
#
# Device algorithm (per core, 4 units = 2 pairs of heads), all matmuls
# fp32r (full PE rate at N>=256, fp32 accumulation). Triangular software
# pipeline over 512-wide s-block "rounds"; for round r:
#   - DMA hidden^T slice; project V (natural [s, feat] layout, stored as V'
#     with fused ones columns), K^T and Q^T (pair-packed [128, s]).
#   - run every attention cell (qb, kc) with max(qb, kc) == r; a cell is:
#     per pair, 4 key chunks of: S^T = K^T-chunk^T Q^T (two heads
#     row-packed, K=64 each), P^T = exp(S^T/8) (mostly on ACT; a tunable
#     fraction of chunks runs on DVE via the Schraudolph exp bit-trick so
#     ACT doesn't pace the PE during cell-dense stretches), O' += V'^T P^T
#     (M=65: psum rows 0-63 unnormalized out^T, row 64 = softmax
#     denominator via the ones column); O' accumulates across rounds in
#     SBUF.
#   - a cell per round is deferred and interleaved with the next round's
#     projections so the ACT engine stays fed.
# Finalize per q-block (interleaved with the last round's cells):
# reciprocal the denominators into a [2, 512] tile (DVE), broadcast both
# units' recips across partitions with ONE K=2 selector matmul (partitions
# 0-63 get unit A's recip, 64-127 unit B's), normalize with two DVE
# multiplies into a unit-stacked [128, 512] O tile, then output projection
# with K=128 (both units contracted per matmul — half the matmuls of the
# per-unit K=64 form) and DMA out.
#
# PSUM plan (8 banks): S^T tiles 2x[128,1024], PV accumulators 2x[128,512],
# projection/broadcast/output transients 2x[128,512] - dedicated pools so
# the streams don't steal each other's slots.
import os
import sys

import numpy as np

try:
    import concourse.bass as bass
except ImportError:  # harness runs from a fresh dir; repo is on the default path
    sys.path.insert(0, "/opt/trn_rl_repo")
    import concourse.bass as bass

import concourse.bacc as bacc
import concourse.mybir as mybir
import concourse.tile as tile
from concourse.bass import ts, ds
from contextlib import ExitStack

B, S, D = 2, 2048, 1024
HEADS, DIM_HEAD = 16, 64
SCALE = DIM_HEAD**-0.5
N_CORES = 8
UNITS = 4  # (b, h) units per core
PAIRS = 2  # head pairs per core
P = 128
SB = S // 512  # 4 s-blocks of 512
QB = S // 512  # 4 q-blocks of 512
DC = D // P  # 8 contraction chunks for projections
KI = S // P  # 16 key chunks of 128
F32 = mybir.dt.float32
F32R = mybir.dt.float32r
I32 = mybir.dt.int32

# Schraudolph exp-as-bf16-bits trick: exp(x*SCALE) ~= bitcast_bf16(
# int16(A*x + B)). A folds the softmax scale and log2(e) into the bf16
# exponent field; B is the bf16 exponent bias minus the minimax shift
# (~0.0579*2^7) that bounds the relative error of the linear-mantissa
# approximation at ~3%, plus 0.5 to center the trunc-to-int. The bf16
# (not f32r) output dtype keeps walrus' "fp32r consumers need fp32r-
# rounded producers" verifier happy; PV matmuls take the bf16 moving
# operand at the same full PE rate. Only a small tunable fraction of exp
# chunks takes this path (error contribution ~f*3%), relieving ACT.
SCH_A = float(2.0**7 * np.log2(np.e) * SCALE)
SCH_B = float(127 * 2**7 - 0.0579 * 2**7 + 0.5)
BF16 = mybir.dt.bfloat16
I16 = mybir.dt.int16


def build_nc():
    nc = bacc.Bacc("TRN2", target_bir_lowering=False, debug=False)

    hiddent = nc.dram_tensor("hiddent", [P, DC, S], F32R, kind="ExternalInput").ap()
    # weights arrive pre-permuted to the on-chip layout (see shard_inputs),
    # declared fp32r end-to-end so the PE takes the full-rate matmul path
    wqt = nc.dram_tensor("wqt", [P, DC, 256], F32R, kind="ExternalInput").ap()
    wkt = nc.dram_tensor("wkt", [P, DC, 256], F32R, kind="ExternalInput").ap()
    wvt = nc.dram_tensor("wvt", [P, DC, 256], F32R, kind="ExternalInput").ap()
    wot = nc.dram_tensor("wot", [P, PAIRS, D], F32R, kind="ExternalInput").ap()
    y = nc.dram_tensor("y", [S, D], F32, kind="ExternalOutput").ap()

    n_xdve = int(os.environ.get("K_XDVE", "8"))  # every-Nth exp chunk -> DVE
    n_xdve_last = int(os.environ.get("K_XDVE_LAST", "0"))

    with tile.TileContext(nc) as tc, ExitStack() as ctx:
        persist = ctx.enter_context(tc.tile_pool(name="persist", bufs=1))
        pt_pool = ctx.enter_context(
            tc.tile_pool(name="pt", bufs=int(os.environ.get("K_PT", "6")))
        )
        otsb_pool = ctx.enter_context(
            tc.tile_pool(name="otsb", bufs=int(os.environ.get("K_OTSB", "4")))
        )
        rc_pool = ctx.enter_context(tc.tile_pool(name="rc", bufs=2))
        # deep enough that drain-phase staging never waits on a y store's
        # full DMA round-trip (~2.3us) before reusing a buffer
        y_pool = ctx.enter_context(
            tc.tile_pool(name="ysb", bufs=int(os.environ.get("K_Y", "6")))
        )
        # PSUM: "st" [128,1024] x2 = 4 banks (S^T tiles); "ot" [128,512] x2 +
        # "cell" [128,512] x2 = 4 banks (projection/broadcast/output
        # transients and PV accumulators).
        st_ps = ctx.enter_context(tc.tile_pool(name="stps", bufs=int(os.environ.get("K_ST", "2")), space="PSUM"))
        ot_ps = ctx.enter_context(
            tc.tile_pool(name="otps", bufs=int(os.environ.get("K_OT", "2")), space="PSUM")
        )
        cell_ps = ctx.enter_context(
            tc.tile_pool(name="cellps", bufs=int(os.environ.get("K_CELL", "2")), space="PSUM")
        )

        # ---- persistent SBUF tensors ----
        KT = persist.tile([P, PAIRS, S], F32R)  # K^T pair-packed
        QT = persist.tile([P, PAIRS, S], F32R)  # Q^T pair-packed
        # V' per (k-chunk, pair): [V_unitA(64) | 1 | V_unitB(64) | 1];
        # each unit's PV is M=65 at base 0: out rows 0-63, sum at row 64.
        # bf16 (not fp32r): PV pairs with the bf16 P^T tiles — the PE rejects
        # mixed 32/16-bit matmul inputs — at the same full rate, and bf16's
        # ~0.2% RMS quantization of V is far inside the 2e-2 gate.
        Vp = persist.tile([P, KI, PAIRS, 130], BF16)
        wq_sb = persist.tile([P, DC, 256], F32R)
        wk_sb = persist.tile([P, DC, 256], F32R)
        wv_a = persist.tile([P, 4, 256], F32R)
        wv_b = persist.tile([P, 4, 256], F32R)
        # Wo pair-stacked: partitions 0-63 = unit 2p rows, 64-127 = unit 2p+1
        wo_sb = persist.tile([P, PAIRS, D], F32R)
        # recip-broadcast operands. DVE/PE partition bases must be 32-aligned,
        # so the two per-unit recips live at partitions 0 and 32 of a [33,512]
        # tile (rows 1-31 stay at the memset value and are zeroed out of the
        # contraction by sel2, whose only nonzero rows are 0 and 32):
        # out[m] = rc[0] for m<64 else rc[32].
        sel2 = persist.tile([33, P], F32R)
        rc2 = persist.tile([33, 2, PAIRS, 512], F32R)  # dim1: qb%2 slot
        # O' accumulator: rows 0-63 unnormalized out^T, row 64 = denominator
        acc = persist.tile([65, QB, PAIRS, 2, 512], F32)
        # hidden^T lives only within its round
        ht_pool = ctx.enter_context(tc.tile_pool(name="htp", bufs=int(os.environ.get("K_HT", "2"))))

        # memset can't write fp32r; stage ones in f32 and round via copies
        ones_f32 = persist.tile([P, P], F32)
        nc.vector.memset(ones_f32, 1.0)
        sel_f32 = persist.tile([33, P], F32)
        nc.vector.memset(sel_f32, 0.0)
        nc.vector.tensor_copy(sel_f32[0:1, 0:64], ones_f32[0:1, 0:64])
        nc.vector.tensor_copy(sel_f32[32:33, 64:128], ones_f32[32:33, 0:64])
        nc.vector.tensor_copy(sel2, sel_f32)
        # keep rc2's never-written rows finite so sel2's zeros annihilate them
        rc2_f32 = persist.tile([33, 2, PAIRS, 512], F32)
        nc.vector.memset(rc2_f32, 0.0)
        nc.vector.tensor_copy(rc2, rc2_f32)
        # weight DMAs split across the three DGE queues by first use so no
        # single queue serializes the startup: V weights + Wq + Wo on the
        # gpsimd SWDGE queue, Wk on the ACT HWDGE queue (ACT is idle at
        # t=0), hidden tiles on the SP queue.
        nc.gpsimd.dma_start(wv_a, wvt[:, 0:4, :])
        nc.gpsimd.dma_start(wv_b, wvt[:, 4:8, :])
        nc.gpsimd.dma_start(wq_sb, wqt)
        nc.gpsimd.dma_start(wo_sb, wot)
        nc.scalar.dma_start(wk_sb, wkt)
        for col in (64, 129):
            nc.vector.tensor_copy(
                Vp[:, :, :, col : col + 1],
                ones_f32[:, 0:32].rearrange("p (a b c) -> p a b c", a=KI, b=PAIRS),
            )

        exp_counter = [0]
        chain_tiles = {}  # (qb, p) -> (otA, otB) for multi-cell PSUM chains

        def attend_cell(qb, kc, pairs=None, xn=None, first=True, last=True,
                        keep_psum=False):
            """Attention for q-block qb against key chunks 4*kc..4*kc+3.

            first/last: diagonal cells of a round share one PSUM accumulation
            group across kc (the PE accumulates for free), so only the
            chain's last cell touches the SBUF accumulator — one copy/add
            per chain instead of one per cell.
            """
            xn_eff = xn if xn is not None else n_xdve
            for p in pairs if pairs is not None else range(PAIRS):
                if first:
                    otA = cell_ps.tile([P, 512], F32, tag="ot")
                    otB = cell_ps.tile([P, 512], F32, tag="ot")
                    chain_tiles[(qb, p)] = (otA, otB)
                else:
                    otA, otB = chain_tiles[(qb, p)]
                for k4 in range(4):
                    ki = kc * 4 + k4
                    stt = st_ps.tile([P, 1024], F32, tag="st")
                    nc.tensor.matmul(
                        stt[:, 0:512],
                        KT[0:64, p, ts(ki, 128)],
                        QT[0:64, p, ts(qb, 512)],
                        start=True,
                        stop=True,
                    )
                    nc.tensor.matmul(
                        stt[:, 512:1024],
                        KT[64:128, p, ts(ki, 128)],
                        QT[64:128, p, ts(qb, 512)],
                        start=True,
                        stop=True,
                    )
                    pt = pt_pool.tile([P, 1024], BF16)
                    exp_counter[0] += 1
                    if xn_eff and exp_counter[0] % xn_eff == 0:
                        # Schraudolph exp on DVE (int16 bf16-bit construction)
                        with nc.allow_low_precision(
                            reason="schraudolph exp, ~3% max rel err on a "
                            "small fraction of chunks"
                        ):
                            nc.vector.tensor_scalar(
                                pt.bitcast(I16),
                                stt,
                                SCH_A,
                                SCH_B,
                                mybir.AluOpType.mult,
                                mybir.AluOpType.add,
                            )
                    else:
                        nc.scalar.activation(
                            pt, stt, mybir.ActivationFunctionType.Exp, scale=SCALE
                        )
                    nc.tensor.matmul(
                        otA[0:65, :],
                        Vp[:, ki, p, 0:65],
                        pt[:, 0:512],
                        start=(k4 == 0 and first),
                        stop=(k4 == 3 and last),
                    )
                    nc.tensor.matmul(
                        otB[0:65, :],
                        Vp[:, ki, p, 65:130],
                        pt[:, 512:1024],
                        start=(k4 == 0 and first),
                        stop=(k4 == 3 and last),
                    )
                if last and not keep_psum:
                    for u, ot in ((0, otA), (1, otB)):
                        sl = acc[:, qb, p, u, :]
                        if first and kc == 0:
                            # single-cell chain at kc 0: plain copy
                            nc.vector.tensor_copy(sl, ot[0:65, :])
                        elif not first:
                            # diagonal chain starting at kc 0: copy replaces
                            # the whole add series
                            nc.vector.tensor_copy(sl, ot[0:65, :])
                        else:
                            nc.vector.tensor_add(sl, sl, ot[0:65, :])

        def fin_srcs(qb, p, from_psum):
            """Unit A/B accumulator views: SBUF acc, or the still-live PSUM
            chain tiles for the kernel's final chain (skips its acc copies)."""
            if from_psum:
                otA, otB = chain_tiles[(qb, p)]
                return otA, otB
            return acc[:, qb, p, 0, :], acc[:, qb, p, 1, :]

        def finalize_recips(qb, p, from_psum=False):
            """Stage 1 (DVE only): pair p's per-unit denominator reciprocals."""
            sl = qb % 2
            sA, sB = fin_srcs(qb, p, from_psum)
            with nc.allow_low_precision(
                reason="fp32r rounding of softmax scale is plenty"
            ):
                nc.vector.reciprocal(rc2[0:1, sl, p, :], sA[64:65, :])
                nc.vector.reciprocal(rc2[32:33, sl, p, :], sB[64:65, :])

        def finalize_norm(qb, p, from_psum=False):
            """Stage 2: recip broadcast (one K=33 selector matmul: partitions
            0-63 <- recip_A, 64-127 <- recip_B) + DVE normalize into a
            unit-stacked [128, 512] O tile for pair p."""
            sl = qb % 2
            sA, sB = fin_srcs(qb, p, from_psum)
            bcp = ot_ps.tile([P, 512], F32, tag="ot")
            nc.tensor.matmul(bcp, sel2, rc2[:, sl, p, :], start=True, stop=True)
            if from_psum:
                # DVE can read only one PSUM operand; stage the broadcast in
                # SBUF (on the otherwise-idle ACT) since sA/sB are PSUM here
                bcs = otsb_pool.tile([P, 512], F32)
                nc.scalar.copy(bcs, bcp)
                bcp = bcs
            otu = otsb_pool.tile([P, 512], F32R)
            nc.vector.tensor_mul(otu[0:64, :], sA[0:64, :], bcp[0:64, :])
            nc.vector.tensor_mul(otu[64:128, :], sB[0:64, :], bcp[64:128, :])
            return otu

        def finalize_outproj(qb, ot_pairs, groups, drain=False):
            """Stage 3 (PE heavy): output projection + DMA out for the given
            (qt, oh) groups. In the drain (no cells left) the PSUM->SBUF
            staging copies alternate ACT/DVE and the stores alternate the
            SP/ACT DMA queues so no single engine serializes the tail."""
            for gi, (qt_i, oh) in enumerate(groups):
                yps = ot_ps.tile([P, 512], F32, tag="ot")
                for pi in range(PAIRS):
                    nc.tensor.matmul(
                        yps,
                        ot_pairs[pi][:, ts(qt_i, 128)],
                        wo_sb[:, pi, ds(oh * 512, 512)],
                        start=(pi == 0),
                        stop=(pi == PAIRS - 1),
                    )
                ysb = y_pool.tile([P, 512], F32)
                ydst = y[qb * 512 + qt_i * 128 : qb * 512 + (qt_i + 1) * 128,
                         ds(oh * 512, 512)]
                if drain and gi % 2 == 0:
                    nc.scalar.copy(ysb, yps)
                    nc.scalar.dma_start(ydst, ysb)
                else:
                    nc.vector.tensor_copy(ysb, yps)
                    nc.sync.dma_start(ydst, ysb)

        GROUPS_A = [(0, 0), (0, 1), (1, 0), (1, 1)]
        GROUPS_B = [(2, 0), (2, 1), (3, 0), (3, 1)]

        def finalize(qb):
            for p in range(PAIRS):
                finalize_recips(qb, p)
            ot_pairs = [finalize_norm(qb, p) for p in range(PAIRS)]
            finalize_outproj(qb, ot_pairs, GROUPS_A + GROUPS_B)

        # ---- triangular pipeline: per s-block round, transpose + project,
        # then run every attention cell that just became ready ----
        deferred = []
        for sb in range(SB):
            # two separate half-tiles so the first projection matmuls
            # (dc 0-3) can start as soon as the first half lands (tile
            # pool dependencies are tile-granular)
            hTa = ht_pool.tile([P, 4, 512], F32R, tag="hta")
            hTb = ht_pool.tile([P, 4, 512], F32R, tag="htb")
            nc.sync.dma_start(hTa, hiddent[:, 0:4, ts(sb, 512)])
            nc.sync.dma_start(hTb, hiddent[:, 4:8, ts(sb, 512)])

            def hts(dc):
                return (hTa if dc < 4 else hTb)[:, dc % 4, :]
            for st in range(4):
                s0 = sb * 512 + st * 128
                ki_idx = sb * 4 + st
                # V projection for this s-tile (natural layout, all 4 units)
                vps = ot_ps.tile([P, 512], F32, tag="ot")
                for dc in range(DC):
                    nc.tensor.matmul(
                        vps[:, :256],
                        hts(dc)[:, ts(st, 128)],
                        (wv_a if dc < 4 else wv_b)[:, dc % 4, :],
                        start=(dc == 0),
                        stop=(dc == DC - 1),
                    )
                for p in range(PAIRS):
                    nc.vector.tensor_copy(
                        Vp[:, ki_idx, p, 0:64], vps[:, (2 * p) * 64 : (2 * p + 1) * 64]
                    )
                    nc.vector.tensor_copy(
                        Vp[:, ki_idx, p, 65:129],
                        vps[:, (2 * p + 1) * 64 : (2 * p + 2) * 64],
                    )
            # K^T / Q^T projections for this s-block (pair-packed),
            # interleaved with cells deferred from the previous round so the
            # ACT engine stays fed while the PE runs projections
            projs = [(w, o, p) for (w, o) in ((wk_sb, KT), (wq_sb, QT))
                     for p in range(PAIRS)]
            for i, (w_sb, out_t, p) in enumerate(projs):
                kps = ot_ps.tile([P, 512], F32, tag="ot")
                for dc in range(DC):
                    nc.tensor.matmul(
                        kps,
                        w_sb[:, dc, ts(p, 128)],
                        hts(dc),
                        start=(dc == 0),
                        stop=(dc == DC - 1),
                    )
                nc.vector.tensor_copy(out_t[:, p, ts(sb, 512)], kps)
                if i < len(deferred):
                    dq, dk, dp, dfirst, dlast = deferred[i]
                    attend_cell(dq, dk, dp, first=dfirst, last=dlast)
            deferred = []

            # newly-ready work as per-pair "pieces": off-diagonal cells
            # (qb, sb) as single-cell groups, then this round's diagonal
            # (sb, 0..sb) as a pair-major PSUM chain (one accumulation group
            # per pair spanning all its kc — no per-cell SBUF adds).
            pieces = [(qb, sb, [p], True, True)
                      for qb in range(sb) for p in range(PAIRS)]
            for p in range(PAIRS):
                for kc in range(sb + 1):
                    pieces.append((sb, kc, [p], kc == 0, kc == sb))
            if sb < SB - 1:
                # defer the last N_DEFER pieces to interleave with the next
                # round's projections
                n_defer = min(int(os.environ.get("K_DEFER", "2")), len(pieces), 4)
                deferred = pieces[len(pieces) - n_defer:] if n_defer else []
                for piece in pieces[: len(pieces) - n_defer]:
                    attend_cell(piece[0], piece[1], piece[2],
                                first=piece[3], last=piece[4])
            else:
                # last round: staged finalize via a thunk queue. Recips (DVE
                # only, no PE impact) fire the moment a (qb, pair) completes;
                # PE-bearing stages (norm's bcp matmul, outproj 4-group
                # halves) are queued and popped between pieces so the PE
                # never sits in-order behind an unfinished DVE leg.
                from collections import deque

                # per-(qb, pair) outstanding piece counts
                left = {}
                for piece in pieces:
                    for p in piece[2]:
                        left[(piece[0], p)] = left.get((piece[0], p), 0) + 1
                thunks = deque()
                fin_state = {}  # qb -> {p: otu}

                def push_finalize(qb, p, from_psum=False):
                    finalize_recips(qb, p, from_psum)  # immediate: DVE-only

                    def norm_thunk(drain=False, qb=qb, p=p, fp=from_psum):
                        fin_state.setdefault(qb, {})[p] = finalize_norm(qb, p, fp)

                    thunks.append(norm_thunk)
                    if p == PAIRS - 1:
                        def oa(drain=False, qb=qb):
                            ots = [fin_state[qb][0], fin_state[qb][1]]
                            finalize_outproj(qb, ots, GROUPS_A, drain=drain)

                        def ob(drain=False, qb=qb):
                            ots = [fin_state[qb][0], fin_state[qb][1]]
                            finalize_outproj(qb, ots, GROUPS_B, drain=drain)

                        thunks.append(oa)
                        thunks.append(ob)

                for pi, piece in enumerate(pieces):
                    # the kernel's very last chain finalizes straight from its
                    # PSUM tiles — its SBUF-accumulator copies are pure
                    # staging on the critical tail
                    final_piece = pi == len(pieces) - 1
                    attend_cell(piece[0], piece[1], piece[2], xn=n_xdve_last,
                                first=piece[3], last=piece[4],
                                keep_psum=final_piece)
                    if thunks:
                        thunks.popleft()()
                    for p in piece[2]:
                        left[(piece[0], p)] -= 1
                        if left[(piece[0], p)] == 0:
                            push_finalize(piece[0], p, from_psum=final_piece)
                while thunks:
                    thunks.popleft()(True)
    nc.compile()
    return nc


_NC = None


def get_nc():
    global _NC
    if _NC is None:
        _NC = build_nc()
    return _NC


def shard_inputs(hidden_states, Wq, Wk, Wv, Wo):
    """Per-core input maps. Core c: batch c//4, heads 4*(c%4) .. 4*(c%4)+3."""
    hidden_states = np.asarray(hidden_states, np.float32)
    Wq, Wk, Wv, Wo = (np.asarray(w, np.float32) for w in (Wq, Wk, Wv, Wo))
    in_maps = []
    for c in range(N_CORES):
        b = c // 4
        f0 = (c % 4) * 4 * DIM_HEAD  # first feature row/col of this core's heads
        rows = slice(f0, f0 + UNITS * DIM_HEAD)

        def proj_layout(w):
            # W[rows].T is [D, 256]; on-chip layout is [128, DC, 256]
            return np.ascontiguousarray(
                w[rows, :].T.reshape(DC, P, 256).transpose(1, 0, 2)
            )

        # Wo[:, rows].T is [256, D]; on-chip layout [128, PAIRS, D] stacks
        # each pair's two units on the partition axis (K=128 out-proj)
        wot = np.ascontiguousarray(
            Wo[:, rows].T.reshape(PAIRS, P, D).transpose(1, 0, 2)
        )
        in_maps.append(
            {
                "hiddent": np.ascontiguousarray(
                    hidden_states[b].T.reshape(DC, P, S).transpose(1, 0, 2)
                ),
                "wqt": proj_layout(Wq),
                "wkt": proj_layout(Wk),
                "wvt": proj_layout(Wv),
                "wot": wot,
            }
        )
    return in_maps


def unshard_outputs(results, bo):
    out = np.zeros((B, S, D), np.float32)
    for c, res in enumerate(results):
        out[c // 4] += res["y"]
    out += np.asarray(bo, np.float32)[None, None, :]
    return out


def kernel(hidden_states, Wq, Wk, Wv, Wo, bo, _trace=False):
    from concourse.bass_utils import run_bass_kernel_spmd

    nc = get_nc()
    in_maps = shard_inputs(hidden_states, Wq, Wk, Wv, Wo)
    res = run_bass_kernel_spmd(nc, in_maps, list(range(N_CORES)), trace=_trace)
    out = unshard_outputs(res.results, bo)
    if _trace:
        return out, res
    return out
